# revision 16
# baseline (speedup 1.0000x reference)
"""Neural CDE (RK4, piecewise-constant path derivative) Trainium2 kernel — v4.

Wall time for this kernel is 2048 x (per-stage critical-path latency): the
RK4 recurrence is strictly serial, so only the latency of one stage matters
(extra parallel chains cannot shorten it, they only contend for engines).
This version minimizes that latency for a single chain over all 16 batch
elements per core:

- All matmuls are double-fp16 3-term (W ~= Whi+Wlo, moving x ~= xb+xr;
  Whi.xb + Whi.xr + Wlo.xb in fp32 PSUM, ~22 mantissa bits). The dynamics
  are chaotic (per-step noise amplified ~300x over the 512 steps) so every
  moving operand needs the double representation; fp16 keeps fast-weight-
  load rates (~27ns/load vs ~213ns for fp32's two no-FWL passes).
- A post-schedule pass (dedupe_ldweights) removes LDWEIGHTS that reload
  the immediately-preceding weights, so the hi-term's load serves both its
  xb and xr matmuls.
- Biases are folded into PSUM via tiny fp16-double seed matmuls (off the
  critical path), which makes relu+split a 2-op sequence with a parallel
  join: hb = (ph max 0) cast to fp16, hr = ((ph max 0) - hb); the hb-only
  matmul terms of the next layer start before hr exists.
- State updates write their fp16 cast directly (STT with fp16 out feeds
  the next L1 immediately); the fp32 slot and fp16 residual follow on the
  same queue and join one matmul later.

Layouts as in the bf16 baseline: state feature-major in "split form"
([128,16], value = top64 + bottom64, W1 row-duplicated so L1 folds the
halves); L3 lands x-major in PSUM gp[128, 8x16] (partition p = (z = p%64,
x = 2c + (p>=64))); tanh -> elementwise * [dt*v | -0.001*dt*sum(v)] ->
unit-stride reduce -> RK4 update.
"""

import sys
from contextlib import ExitStack

import numpy as np

sys.path.insert(0, "/opt/trn_rl_repo")

import concourse.bass as bass
import concourse.tile as tile
from concourse import bacc
from concourse import mybir
from concourse.bass_utils import run_bass_kernel_spmd

B, L, X, Z, H = 128, 512, 16, 64, 128
NCORES = 8
BPC = B // NCORES  # 16
DT = 0.1
F32 = mybir.dt.float32
F16 = mybir.dt.float16
AF = mybir.ActivationFunctionType
OP = mybir.AluOpType

_p = np.arange(128)
_c = np.arange(8)
ORIG_COL = (_p[None, :] % 64) * 16 + 2 * _c[:, None] + (_p[None, :] // 64)  # [8,128]


def dedupe_ldweights(nc):
    """Drop InstLdweights identical to the previous one on the PE stream."""
    removed = 0
    for bb in nc.m.functions[0].blocks:
        insts = bb.instructions
        keep = []
        last_key = None
        for i in insts:
            tn = type(i).__name__
            if tn == "InstLdweights":
                a = i.ins[0]
                k = (a.memref, a.offset, str(a.ap), str(i.perf_mode),
                     str(i.is_transpose), str(i.tile_position),
                     str(i.tile_size))
                si = i.sync_info
                free = si is None or (not si.on_wait and not si.on_update)
                if k == last_key and free:
                    removed += 1
                    continue
                last_key = k
            elif tn not in ("InstMatmult", "InstEventSemaphore", "InstNop") \
                    and getattr(i, "engine", None) == mybir.EngineType.PE:
                last_key = None
            keep.append(i)
        insts[:] = keep
    return removed


def build_nc(l_steps=L):
    nc = bacc.Bacc("TRN2")

    dp = nc.declare_dram_parameter
    vsmall = dp("vsmall", [l_steps, 256], F32, isOutput=False).ap()  # dt*v x-major
    svd = dp("svd", [l_steps, 16], F32, isOutput=False).ap()  # -0.001*dt*sum_x v
    # [w1hi|w1lo|w2hi|w2lo|w3hi(1024)|w3lo(1024)] fp16, single DMA
    wmm_d = dp("wmm", [128, 2560], F16, isOutput=False).ap()
    # [[b3hi;b3lo] | [sel;sel]] and [b1hi;b1lo;b2hi;b2lo| ones..]
    b3sel_d = dp("b3sel", [16, 256], F16, isOutput=False).ap()
    b12_d = dp("b12", [2, 288], F16, isOutput=False).ap()  # rows hi,lo; cols [b1(128)|ones(16)|b2(128)|ones(16)]
    wi1x_d = dp("wi1x", [16, 144], F32, isOutput=False).ap()
    wi2_d = dp("wi2", [128, 128], F32, isOutput=False).ap()
    wi3_d = dp("wi3", [128, 64], F32, isOutput=False).ap()
    bi1_d = dp("bi1", [128, 1], F32, isOutput=False).ap()
    bi2_d = dp("bi2", [128, 1], F32, isOutput=False).ap()
    bi3_d = dp("bi3", [64, 1], F32, isOutput=False).ap()
    zall = dp("zall", [l_steps, 128, BPC], F32, isOutput=True).ap()

    with tile.TileContext(nc) as tc, ExitStack() as ctx:
        singles = ctx.enter_context(tc.tile_pool(name="singles", bufs=1))
        wfp = ctx.enter_context(tc.tile_pool(name="wfp", bufs=3))
        gep = ctx.enter_context(tc.tile_pool(name="gep", bufs=4))
        hp = ctx.enter_context(tc.tile_pool(name="hp", bufs=2))
        zp = ctx.enter_context(tc.tile_pool(name="zp", bufs=2))
        qp = ctx.enter_context(tc.tile_pool(name="qp", bufs=3))
        kp = ctx.enter_context(tc.tile_pool(name="kp", bufs=2))
        mp = ctx.enter_context(tc.tile_pool(name="mp", bufs=2))
        ph1p = ctx.enter_context(tc.tile_pool(name="ph1p", bufs=2, space="PSUM"))
        ph2p = ctx.enter_context(tc.tile_pool(name="ph2p", bufs=2, space="PSUM"))
        gpp = ctx.enter_context(tc.tile_pool(name="gpp", bufs=2, space="PSUM"))
        dummyp = ctx.enter_context(tc.tile_pool(name="dummyp", bufs=1, space="PSUM"))

        dma = nc.sync.dma_start
        mm = nc.tensor.matmul

        def load(pool, ap):
            t = pool.tile(list(ap.shape), ap.dtype, tag=ap.tensor.name,
                          name=ap.tensor.name)
            dma(out=t[:], in_=ap)
            return t

        wmm = load(singles, wmm_d)
        w1hi, w1lo = wmm[:, 0:128], wmm[:, 128:256]
        w2hi, w2lo = wmm[:, 256:384], wmm[:, 384:512]
        w3hi, w3lo = wmm[:, 512:1536], wmm[:, 1536:2560]
        b3sel = load(singles, b3sel_d)
        b3d, sel16 = b3sel[:, 0:128], b3sel[:, 128:256]
        b12 = load(singles, b12_d)
        b1d, b2d = b12[:, 0:128], b12[:, 144:272]
        ones1, ones2 = b12[:, 128:144], b12[:, 272:288]
        wi1x = load(singles, wi1x_d)
        wi1, x0t = wi1x[:, 0:128], wi1x[:, 128:144]
        wi2 = load(singles, wi2_d)
        wi3 = load(singles, wi3_d)
        bi1 = load(singles, bi1_d)
        bi2 = load(singles, bi2_d)
        bi3 = load(singles, bi3_d)

        # ---- init MLP (fp32): z0 = mlp(x(t0)) ----
        gi1 = gpp.tile([128, 128], F32, tag="gp")
        mm(gi1[:, 0:16], wi1, x0t, start=True, stop=True, skip_group_check=True)
        hi1 = singles.tile([128, 16], F32, tag="hi1")
        nc.scalar.activation(hi1[:], gi1[:, 0:16], AF.Relu, bias=bi1[:])
        gi2 = gpp.tile([128, 128], F32, tag="gp")
        mm(gi2[:, 0:16], wi2[:], hi1[:], start=True, stop=True,
           skip_group_check=True)
        hi2 = singles.tile([128, 16], F32, tag="hi2")
        nc.scalar.activation(hi2[:], gi2[:, 0:16], AF.Relu, bias=bi2[:])
        gi3 = gpp.tile([128, 128], F32, tag="gp")
        mm(gi3[0:64, 0:16], wi3[:], hi2[:], start=True, stop=True,
           skip_group_check=True)

        ge_s = gep.tile([128, 144], F32, tag="ge", name="ge0")
        slot0 = ge_s[:, 128:144]
        nc.vector.tensor_scalar_add(ge_s[0:64, 128:144], gi3[0:64, 0:16],
                                    bi3[:])
        nc.vector.memset(ge_s[64:128, 128:144], 0.0)
        zb_cur = zp.tile([128, BPC], F16, tag="zb", name="zb0")
        nc.vector.tensor_copy(out=zb_cur[:], in_=slot0)
        zr_cur = zp.tile([128, BPC], F16, tag="zr", name="zr0")
        nc.vector.scalar_tensor_tensor(out=zr_cur[:], in0=zb_cur[:],
                                       scalar=-1.0, in1=slot0,
                                       op0=OP.mult, op1=OP.add)

        def issue_wf(t):
            wf = wfp.tile([128, 144], F32, tag="wf", name="wf")
            vbase = vsmall[t]
            for half in range(2):
                src = bass.AP(
                    tensor=vbase.tensor,
                    offset=vbase.offset + 16 * half,
                    ap=[[0, 64], [32, 8], [1, 16]],
                )
                dst = wf[64 * half:64 * (half + 1), 0:128].rearrange(
                    "p (c j) -> p c j", j=16)
                dma(out=dst, in_=src)
            sbase = svd[t]
            src = bass.AP(tensor=sbase.tensor, offset=sbase.offset,
                          ap=[[0, 128], [1, 16]])
            dma(out=wf[:, 128:144], in_=src)
            return wf

        wf_cur = issue_wf(0)
        stage_scale = [0.5, 0.5, 1.0]

        for t in range(l_steps):
            wf_next = issue_wf(t + 1) if t + 1 < l_steps else None
            dma(out=zall[t], in_=slot0)

            qs = []
            kacc12 = kacc123 = pfin = None
            for s in range(4):
                # ---- L1: b1 seed + 3-term state matmuls ----
                ph1 = ph1p.tile([128, BPC], F32, tag="ph1", name="ph1")
                mm(ph1[:], b1d, ones1, start=True, stop=False,
                   skip_group_check=True)
                mm(ph1[:], w1hi, zb_cur[:], start=False, stop=False,
                   skip_group_check=True)
                mm(ph1[:], w1hi, zr_cur[:], start=False, stop=False,
                   skip_group_check=True)
                mm(ph1[:], w1lo, zb_cur[:], start=False, stop=True,
                   skip_group_check=True)
                # relu1: fp16 value first (feeds L2), residual joins late
                h1b = hp.tile([128, BPC], F16, tag="h1b", name="h1b")
                nc.vector.tensor_scalar(h1b[:], ph1[:], 0.0, None, OP.max)
                h1r = hp.tile([128, BPC], F16, tag="h1r", name="h1r")
                nc.vector.scalar_tensor_tensor(
                    out=h1r[:], in0=ph1[:], scalar=0.0, in1=h1b[:],
                    op0=OP.max, op1=OP.subtract)

                # ---- L2 ----
                ph2 = ph2p.tile([128, BPC], F32, tag="ph2", name="ph2")
                mm(ph2[:], b2d, ones2, start=True, stop=False,
                   skip_group_check=True)
                mm(ph2[:], w2hi, h1b[:], start=False, stop=False,
                   skip_group_check=True)
                mm(ph2[:], w2lo, h1b[:], start=False, stop=False,
                   skip_group_check=True)
                mm(ph2[:], w2hi, h1r[:], start=False, stop=True,
                   skip_group_check=True)
                h2b = hp.tile([128, BPC], F16, tag="h2b", name="h2b")
                nc.vector.tensor_scalar(h2b[:], ph2[:], 0.0, None, OP.max)
                h2r = hp.tile([128, BPC], F16, tag="h2r", name="h2r")
                nc.vector.scalar_tensor_tensor(
                    out=h2r[:], in0=ph2[:], scalar=0.0, in1=h2b[:],
                    op0=OP.max, op1=OP.subtract)

                # ---- L3: b3 seed + 8 x-major chunks, hb terms first ----
                gp = gpp.tile([128, 128], F32, tag="gp", name="gp")
                mm(gp[:], b3d, sel16, start=True, stop=False,
                   skip_group_check=True)
                for c in range(8):
                    sl = gp[:, c * 16:(c + 1) * 16]
                    whi = w3hi[:, c * 128:(c + 1) * 128]
                    wlo = w3lo[:, c * 128:(c + 1) * 128]
                    mm(sl, whi, h2b[:], start=False, stop=False,
                       skip_group_check=True)
                    mm(sl, whi, h2r[:], start=False, stop=False,
                       skip_group_check=True)
                    mm(sl, wlo, h2b[:], start=False, stop=(c == 7),
                       skip_group_check=True)

                # PE warm-keeping: wide dummy matmuls fill the tanh/mult/
                # reduce window so the PE clock stays ramped and the next
                # stage's small matmuls avoid the cold ~172ns fill cost.
                dum = dummyp.tile([128, 512], F32, tag="dum", name="dum")
                mm(dum[:], w1hi, wmm[:, 512:1024], start=True, stop=True,
                   skip_group_check=True)
                mm(dum[:], w1hi, wmm[:, 1024:1536], start=True, stop=True,
                   skip_group_check=True)
                nc.scalar.activation(ge_s[:, 0:128], gp[:], AF.Tanh,
                                     bias=0.0)
                m = mp.tile([128, 144], F32, tag="m", name="m")
                nc.vector.tensor_tensor(
                    out=m[:].rearrange("p (j c) -> p j c", c=9),
                    in0=ge_s[:, 0:144].rearrange("p (c j) -> p j c", j=16),
                    in1=wf_cur[:].rearrange("p (c j) -> p j c", j=16),
                    op=OP.mult,
                )
                q = qp.tile([128, BPC], F32, tag="q", name="q")
                nc.vector.tensor_reduce(
                    out=q[:], in_=m[:].rearrange("p (j c) -> p j c", c=9),
                    axis=mybir.AxisListType.X, op=OP.add,
                )
                qs.append(q)

                last = t == l_steps - 1 and s == 3
                if not last:
                    ge_n = gep.tile([128, 144], F32, tag="ge", name="ge")
                    zb_n = zp.tile([128, BPC], F16, tag="zb", name="zb")
                    zr_n = zp.tile([128, BPC], F16, tag="zr", name="zr")
                    if s < 3:
                        # fp16 state first (feeds next L1), fp32 + residual follow
                        nc.vector.scalar_tensor_tensor(
                            out=zb_n[:], in0=q[:], scalar=stage_scale[s],
                            in1=slot0, op0=OP.mult, op1=OP.add)
                        nc.vector.scalar_tensor_tensor(
                            out=ge_n[:, 128:144], in0=q[:],
                            scalar=stage_scale[s], in1=slot0,
                            op0=OP.mult, op1=OP.add)
                    else:
                        nc.vector.scalar_tensor_tensor(
                            out=zb_n[:], in0=q[:], scalar=1.0 / 6.0,
                            in1=pfin[:], op0=OP.mult, op1=OP.add)
                        nc.vector.scalar_tensor_tensor(
                            out=ge_n[:, 128:144], in0=q[:], scalar=1.0 / 6.0,
                            in1=pfin[:], op0=OP.mult, op1=OP.add)
                    nc.vector.scalar_tensor_tensor(
                        out=zr_n[:], in0=zb_n[:], scalar=-1.0,
                        in1=ge_n[:, 128:144], op0=OP.mult, op1=OP.add)
                    ge_s = ge_n
                    if s == 3:
                        slot0 = ge_n[:, 128:144]
                    zb_cur, zr_cur = zb_n, zr_n
                # k accumulators (off the critical path)
                if s == 1:
                    kacc12 = kp.tile([128, BPC], F32, tag="k12", name="k12")
                    nc.vector.scalar_tensor_tensor(
                        out=kacc12[:], in0=q[:], scalar=2.0, in1=qs[0][:],
                        op0=OP.mult, op1=OP.add)
                elif s == 2:
                    kacc123 = kp.tile([128, BPC], F32, tag="k123",
                                      name="k123")
                    nc.vector.scalar_tensor_tensor(
                        out=kacc123[:], in0=q[:], scalar=2.0, in1=kacc12[:],
                        op0=OP.mult, op1=OP.add)
                    pfin = kp.tile([128, BPC], F32, tag="pf", name="pf")
                    nc.vector.scalar_tensor_tensor(
                        out=pfin[:], in0=kacc123[:], scalar=1.0 / 6.0,
                        in1=slot0, op0=OP.mult, op1=OP.add)
            wf_cur = wf_next

    n = dedupe_ldweights(nc)
    nc.compile()
    return nc


def _split16(w):
    w = np.asarray(w, np.float64)
    hi = w.astype(np.float16)
    lo = (w - hi.astype(np.float64)).astype(np.float16)
    return hi, lo


def _prep_inputs(t, x, dyn_w1, dyn_b1, dyn_w2, dyn_b2, dyn_w3, dyn_b3,
                 init_w1, init_b1, init_w2, init_b2, init_w3, init_b3,
                 l_steps=L):
    x = np.asarray(x, dtype=np.float32)
    x_aug = np.concatenate([x, x[:, -1:]], axis=1)
    v = (x_aug[:, 1:] - x_aug[:, :-1]) / DT
    sv = v.sum(-1)

    w1s = np.concatenate([dyn_w1, dyn_w1], axis=0).astype(np.float64)
    w3x = np.empty((H, 1024), dtype=np.float64)
    for c in range(8):
        w3x[:, c * 128:(c + 1) * 128] = np.asarray(dyn_w3, np.float64)[:, ORIG_COL[c]]
    w1hi, w1lo = _split16(w1s)
    w2hi, w2lo = _split16(dyn_w2)
    w3hi, w3lo = _split16(w3x)
    wmm = np.concatenate([w1hi, w1lo, w2hi, w2lo, w3hi, w3lo], axis=1)

    b3row = np.asarray(dyn_b3, np.float64)[ORIG_COL]
    b3hi, b3lo = _split16(b3row)
    sel = np.repeat(np.eye(8, dtype=np.float16), 16, axis=1)  # [8,128]
    b3sel = np.zeros((16, 256), dtype=np.float16)
    b3sel[0:8, 0:128] = b3hi
    b3sel[8:16, 0:128] = b3lo
    b3sel[0:8, 128:256] = sel
    b3sel[8:16, 128:256] = sel

    b1hi, b1lo = _split16(np.asarray(dyn_b1, np.float64).reshape(1, 128))
    b2hi, b2lo = _split16(np.asarray(dyn_b2, np.float64).reshape(1, 128))
    b12 = np.zeros((2, 288), dtype=np.float16)
    b12[0, 0:128] = b1hi
    b12[1, 0:128] = b1lo
    b12[0, 144:272] = b2hi
    b12[1, 144:272] = b2lo
    b12[:, 128:144] = 1.0
    b12[:, 272:288] = 1.0

    shared = dict(
        wmm=np.ascontiguousarray(wmm),
        b3sel=np.ascontiguousarray(b3sel),
        b12=np.ascontiguousarray(b12),
        wi2=np.asarray(init_w2, np.float32),
        wi3=np.asarray(init_w3, np.float32),
        bi1=np.asarray(init_b1, np.float32).reshape(128, 1),
        bi2=np.asarray(init_b2, np.float32).reshape(128, 1),
        bi3=np.asarray(init_b3, np.float32).reshape(64, 1),
    )
    wi1 = np.asarray(init_w1, np.float32)

    in_maps = []
    for core in range(NCORES):
        sl = slice(core * BPC, (core + 1) * BPC)
        vb = v[sl, :l_steps]
        svb = sv[sl, :l_steps]
        vsm = (DT * vb.transpose(1, 2, 0)).reshape(l_steps, 256).astype(np.float32)
        svdc = (-0.001 * DT * svb.T).astype(np.float32)
        x0tc = x[sl, 0, :].T.astype(np.float32)
        wi1x = np.concatenate([wi1, x0tc], axis=1)
        m = dict(shared)
        m.update(vsmall=np.ascontiguousarray(vsm),
                 svd=np.ascontiguousarray(svdc),
                 wi1x=np.ascontiguousarray(wi1x))
        in_maps.append(m)
    return in_maps


_NC_CACHE = {}


def kernel_traced(trace=False, tmpdir=None, **inputs):
    key = L
    if key not in _NC_CACHE:
        _NC_CACHE[key] = build_nc(L)
    nc = _NC_CACHE[key]
    in_maps = _prep_inputs(**inputs, l_steps=L)
    res = run_bass_kernel_spmd(nc, in_maps, list(range(NCORES)), trace=trace,
                               tmpdir=tmpdir)
    out = np.empty((B, L, Z), dtype=np.float32)
    for core in range(NCORES):
        zf = res.results[core]["zall"]
        out[core * BPC:(core + 1) * BPC] = \
            (zf[:, :Z] + zf[:, Z:]).transpose(2, 0, 1)
    return out, res


def kernel(**inputs):
    return kernel_traced(trace=False, **inputs)[0]
